# revision 10
# baseline (speedup 1.0000x reference)
"""v5: row-major LSTM cell kernel, fp16 I/O, K-stacked pair matmuls.

Sharding: pure data-parallel, batch split 8 ways (131072 rows/core).

Row mapping (per core): row = w2*4096 + 32p + 16a + m, where w2 = window
pair, a = window parity, p = partition, m = chunk-in-window = 2j + half
(j = matmul pair, half = A/B slot). c/hn/cn host arrays are NATURAL
reshapes [nwin/2, 128, 2*16*32] of the row-major [R, 32] arrays.

Host prep (layout only; all compute on device):
  W_pad [128,256] fp16, cols [Ai|Af|Ao|Bi|Bf|Bo|Ag|Bg] (A rows 0:49,
    B rows 64:113 of the stationary; zeros elsewhere kill junk lanes)
  xha/xhb [nwin, 49, 8*128] fp16: window w=2*w2+a, col j*128+p =
    [x|h|1](row(w2, a, p, m=2j+half)), half=0 for xha, 1 for xhb

Device, per 2048-row window:
  - xh_sb [128, 8, 128]: parts 0:49 <- xha, 64:113 <- xhb
  - 8 matmuls: lhsT = xh_sb[:, j, :] stationary, rhs = W_pad [128, 256]
    -> PSUM [128, 8, 256] f32
  - ACT: sigmoid PSUM[:, :, 0:192] -> sfo fp16; tanh [:, :, 192:256] -> g
  - DVE (fp16 SBUF 2x): m1=i*g, m2=f*c, cn=m1+m2, hn=o*tanh(cn)
  - ACT: tc=tanh(cn)
  - c in on sync; hn/cn out on gpsimd, batched per 2 windows
"""

import sys

if "/opt/trn_rl_repo" not in sys.path:
    sys.path.insert(0, "/opt/trn_rl_repo")

import ml_dtypes
import numpy as np

import bass_rust
import concourse.bass as bass
import concourse.tile as tile
from concourse import mybir

F32 = mybir.dt.float32
F16 = mybir.dt.float16
BF16 = mybir.dt.bfloat16
AF = mybir.ActivationFunctionType

B = 1048576
N_CORES = 8
R = B // N_CORES
IN_DIM, H_DIM = 16, 32
XH = IN_DIM + H_DIM
K_AUG = XH + 1  # 49
G4 = 4 * H_DIM  # 128
P = 128
NPAIR = 8  # matmul pairs per window
CH = 2 * NPAIR  # 16 chunks (of 128 rows) per window
WIN = CH * P  # 2048 rows per window
NWIN = R // WIN  # 64


def _split_waits(nc, max_waits=1):
    """Walrus codegen allows at most one semaphore wait per instruction."""
    n = 0
    for f in nc.m.functions:
        for blk in f.blocks:
            insts = blk.instructions
            new = []
            for inst in insts:
                si = inst.sync_info
                waits = list(si.on_wait) if si and si.on_wait else []
                if len(waits) > max_waits:
                    excess, keep = waits[:-max_waits], waits[-max_waits:]
                    for j in range(0, len(excess), max_waits):
                        nop = mybir.InstEventSemaphore(
                            name=f"{inst.name}-tw{j}", ins=[], outs=[]
                        )
                        nop.engine = inst.engine
                        nop.sync_info = bass_rust.SyncInfo(
                            on_wait=excess[j : j + max_waits], on_update=[]
                        )
                        new.append(nop)
                        n += 1
                    si.on_wait = keep
                    inst.sync_info = si
                new.append(inst)
            insts[:] = new
    return n


def build_nc(rows=R):
    assert rows % (2 * WIN) == 0
    nwin = rows // WIN

    nc = bass.Bass()
    xha = nc.dram_tensor("xha", [nwin, K_AUG, NPAIR * P], F16, kind="ExternalInput")
    xhb = nc.dram_tensor("xhb", [nwin, K_AUG, NPAIR * P], F16, kind="ExternalInput")
    c_in = nc.dram_tensor(
        "c_in", [nwin // 2, P, 2 * CH * H_DIM], F16, kind="ExternalInput"
    )
    w = nc.dram_tensor("w", [P, 2 * G4], F16, kind="ExternalInput")
    hn_out = nc.dram_tensor(
        "hn", [nwin // 2, P, 2 * CH * H_DIM], F16, kind="ExternalOutput"
    )
    cn_out = nc.dram_tensor(
        "cn", [nwin // 2, P, 2 * CH * H_DIM], F16, kind="ExternalOutput"
    )

    with tile.TileContext(nc) as tc:
        with (
            tc.tile_pool(name="const", bufs=1) as constp,
            tc.tile_pool(name="io", bufs=3) as iop,
            tc.tile_pool(name="pair", bufs=2) as pairp,
            tc.tile_pool(name="work", bufs=3) as workp,
            tc.tile_pool(name="psum", bufs=2, space="PSUM") as psump,
        ):
            w_sb = constp.tile([P, 2 * G4], F16, tag="w")
            nc.sync.dma_start(w_sb[:], w[:])

            c_t = None
            hn_t = None
            cn_t = None
            for it in range(nwin):
                half = it % 2
                xh_sb = iop.tile([P, NPAIR, P], F16, tag="xh")
                nc.sync.dma_start(
                    xh_sb[0:K_AUG], xha[it].rearrange("k (j p) -> k j p", j=NPAIR)
                )
                nc.sync.dma_start(
                    xh_sb[K_AUG : 2 * K_AUG],
                    xhb[it].rearrange("k (j p) -> k j p", j=NPAIR),
                )
                if half == 0:
                    c_t = pairp.tile([P, 2, NPAIR, 2, H_DIM], F16, tag="c")
                    nc.sync.dma_start(
                        c_t[:].rearrange("p a j m h -> p (a j m h)"), c_in[it // 2]
                    )
                    hn_t = pairp.tile([P, 2, NPAIR, 2, H_DIM], F16, tag="hn")
                    cn_t = pairp.tile([P, 2, NPAIR, 2, H_DIM], F16, tag="cn")

                ps = psump.tile([P, NPAIR, 2 * G4], F32, tag="ps")
                for j in range(NPAIR):
                    nc.tensor.matmul(
                        ps[:, j, :],
                        xh_sb[0 : 2 * K_AUG, j, :],
                        w_sb[0 : 2 * K_AUG],
                        start=True,
                        stop=True,
                    )

                # sfo free layout per pair j: [Ai|Af|Ao|Bi|Bf|Bo] (6*32)
                sfo = workp.tile([P, NPAIR, 6 * H_DIM], F16, tag="sfo")
                nc.scalar.activation(sfo[:], ps[:, :, 0 : 6 * H_DIM], AF.Sigmoid)
                g_sb = workp.tile([P, NPAIR, 2 * H_DIM], F16, tag="g")
                nc.scalar.activation(
                    g_sb[:], ps[:, :, 6 * H_DIM : 2 * G4], AF.Tanh
                )

                sfo4 = sfo[:].rearrange("p j (m g) -> p j m g", m=2)
                g4 = g_sb[:].rearrange("p j (m h) -> p j m h", m=2)
                c4 = c_t[:, half]
                m1 = workp.tile([P, NPAIR, 2, H_DIM], F16, tag="m1")
                nc.vector.tensor_mul(m1[:], sfo4[:, :, :, 0:H_DIM], g4)
                m2 = workp.tile([P, NPAIR, 2, H_DIM], F16, tag="m2")
                nc.vector.tensor_mul(m2[:], sfo4[:, :, :, H_DIM : 2 * H_DIM], c4)
                nc.vector.tensor_add(cn_t[:, half], m1[:], m2[:])
                tc_sb = workp.tile([P, NPAIR, 2, H_DIM], F16, tag="tc")
                nc.scalar.activation(tc_sb[:], cn_t[:, half], AF.Tanh)
                nc.vector.tensor_mul(
                    hn_t[:, half], sfo4[:, :, :, 2 * H_DIM : 3 * H_DIM], tc_sb[:]
                )

                if half == 1:
                    nc.gpsimd.dma_start(
                        cn_out[it // 2], cn_t[:].rearrange("p a j m h -> p (a j m h)")
                    )
                    nc.gpsimd.dma_start(
                        hn_out[it // 2], hn_t[:].rearrange("p a j m h -> p (a j m h)")
                    )

    _split_waits(nc)
    return nc


def host_prep(x, h, c, Wx, Wh, b):
    """Layout-only host prep for the full batch. Returns fp16 arrays."""
    n = x.shape[0]
    nwin = n // WIN

    A = np.empty((n, K_AUG), dtype=np.float16)
    A[:, 0:IN_DIM] = np.asarray(x, np.float32)
    A[:, IN_DIM:XH] = np.asarray(h, np.float32)
    A[:, XH] = 1.0
    # row = w2*4096 + 32p + 16a + m,  m = 2j + half
    A6 = A.reshape(nwin // 2, P, 2, NPAIR, 2, K_AUG)  # [w2, p, a, j, half, k]
    # window w = 2*w2 + a; col index = j*128 + p
    xha = np.ascontiguousarray(
        A6[:, :, :, :, 0, :].transpose(0, 2, 4, 3, 1).reshape(nwin, K_AUG, NPAIR * P)
    )
    xhb = np.ascontiguousarray(
        A6[:, :, :, :, 1, :].transpose(0, 2, 4, 3, 1).reshape(nwin, K_AUG, NPAIR * P)
    )

    c_host = np.ascontiguousarray(
        np.asarray(c, np.float32)
        .astype(np.float16)
        .reshape(nwin // 2, P, 2 * CH * H_DIM)
    )

    W = np.concatenate(
        [np.asarray(Wx), np.asarray(Wh), np.asarray(b)[None, :]], axis=0
    ).astype(np.float32)  # [49, 128] cols [i|f|g|o]
    Wi, Wf, Wg, Wo = W[:, 0:32], W[:, 32:64], W[:, 64:96], W[:, 96:128]
    Wifo = np.concatenate([Wi, Wf, Wo], axis=1)  # [49, 96]
    w_host = np.zeros((P, 2 * G4), dtype=np.float16)
    w_host[0:K_AUG, 0:96] = Wifo  # A: i f o
    w_host[K_AUG : 2 * K_AUG, 96:192] = Wifo  # B: i f o
    w_host[0:K_AUG, 192:224] = Wg  # A: g
    w_host[K_AUG : 2 * K_AUG, 224:256] = Wg  # B: g
    return xha, xhb, c_host, w_host


_NC_CACHE = {}


def _get_nc(rows=R):
    if rows not in _NC_CACHE:
        _NC_CACHE[rows] = build_nc(rows)
    return _NC_CACHE[rows]


def run(x, h, c, Wx, Wh, b, trace=False, rows=R, n_cores=N_CORES):
    """Shard, execute on the 8 cores, gather. Returns (h_new, c_new, results)."""
    from concourse.bass_utils import run_bass_kernel_spmd

    xha, xhb, c_host, w_host = host_prep(x, h, c, Wx, Wh, b)
    nc = _get_nc(rows)
    nwin = rows // WIN
    in_maps = []
    for i in range(n_cores):
        sl = slice(i * nwin, (i + 1) * nwin)
        sl2 = slice(i * nwin // 2, (i + 1) * nwin // 2)
        in_maps.append(
            {
                "xha": xha[sl],
                "xhb": xhb[sl],
                "c_in": c_host[sl2],
                "w": w_host,
            }
        )
    res = run_bass_kernel_spmd(nc, in_maps, list(range(n_cores)), trace=trace)
    n = rows * n_cores
    h_new = np.empty((n, H_DIM), dtype=np.float32)
    c_new = np.empty((n, H_DIM), dtype=np.float32)
    for i, r in enumerate(res.results):
        sl = slice(i * rows, (i + 1) * rows)
        h_new[sl] = r["hn"].reshape(rows, H_DIM).astype(np.float32)
        c_new[sl] = r["cn"].reshape(rows, H_DIM).astype(np.float32)
    return h_new, c_new, res


def kernel(x, h, c, Wx, Wh, b):
    h_new, c_new, _ = run(x, h, c, Wx, Wh, b)
    return h_new, c_new


# revision 11
# speedup vs baseline: 1.0257x; 1.0257x over previous
"""v7: row-major LSTM cell kernel, fp16 I/O, K-stacked pair matmuls.

Perf history (HW exec, 8 cores): v3 feature-major baseline 550us ->
v4 row-major fp16 261us -> v5 pair-matmuls + DMA batching 199us.
Span is ScalarE-bound (~170us of sigmoid/tanh at 1 elem/lane/cycle --
the 5 transcendentals per output element are the hard floor; GpSimd has
no LUT and PSUM is PE-write/ACT-read only, so no further offload).
bf16 matmul inputs were tried (196us) but push rel err to 1.5e-2 vs
the 2e-2 gate; fp16 keeps it at 2e-3.

Sharding: pure data-parallel, batch split 8 ways (131072 rows/core).

Row mapping (per core): row = w2*4096 + 32p + 16a + m, where w2 = window
pair, a = window parity, p = partition, m = chunk-in-window = 2j + half
(j = matmul pair, half = A/B slot). c/hn/cn host arrays are NATURAL
reshapes [nwin/2, 128, 2*16*32] of the row-major [R, 32] arrays.

Host prep (layout only; all compute on device):
  W_pad [128,256] fp16, cols [Ai|Af|Ao|Bi|Bf|Bo|Ag|Bg] (A rows 0:49,
    B rows 64:113 of the stationary; zeros elsewhere kill junk lanes)
  xha/xhb [nwin, 49, 8*128] fp16: window w=2*w2+a, col j*128+p =
    [x|h|1](row(w2, a, p, m=2j+half)), half=0 for xha, 1 for xhb

Device, per 2048-row window:
  - xh_sb [128, 8, 128]: parts 0:49 <- xha, 64:113 <- xhb
  - 8 matmuls: lhsT = xh_sb[:, j, :] stationary, rhs = W_pad [128, 256]
    -> PSUM [128, 8, 256] f32
  - ACT: sigmoid PSUM[:, :, 0:192] -> sfo fp16; tanh [:, :, 192:256] -> g
  - DVE (fp16 SBUF 2x): m1=i*g, m2=f*c, cn=m1+m2, hn=o*tanh(cn)
  - ACT: tc=tanh(cn)
  - c in on sync; hn/cn out on gpsimd, batched per 2 windows
"""

import sys

if "/opt/trn_rl_repo" not in sys.path:
    sys.path.insert(0, "/opt/trn_rl_repo")

import ml_dtypes
import numpy as np

import bass_rust
import concourse.bass as bass
import concourse.tile as tile
from concourse import mybir

F32 = mybir.dt.float32
F16 = mybir.dt.float16
BF16 = mybir.dt.bfloat16
AF = mybir.ActivationFunctionType

B = 1048576
N_CORES = 8
R = B // N_CORES
IN_DIM, H_DIM = 16, 32
XH = IN_DIM + H_DIM
K_AUG = XH + 1  # 49
G4 = 4 * H_DIM  # 128
P = 128
NPAIR = 8  # matmul pairs per window
CH = 2 * NPAIR  # 16 chunks (of 128 rows) per window
WIN = CH * P  # 2048 rows per window
NWIN = R // WIN  # 64


def _split_waits(nc, max_waits=1):
    """Walrus codegen allows at most one semaphore wait per instruction."""
    n = 0
    for f in nc.m.functions:
        for blk in f.blocks:
            insts = blk.instructions
            new = []
            for inst in insts:
                si = inst.sync_info
                waits = list(si.on_wait) if si and si.on_wait else []
                if len(waits) > max_waits:
                    excess, keep = waits[:-max_waits], waits[-max_waits:]
                    for j in range(0, len(excess), max_waits):
                        nop = mybir.InstEventSemaphore(
                            name=f"{inst.name}-tw{j}", ins=[], outs=[]
                        )
                        nop.engine = inst.engine
                        nop.sync_info = bass_rust.SyncInfo(
                            on_wait=excess[j : j + max_waits], on_update=[]
                        )
                        new.append(nop)
                        n += 1
                    si.on_wait = keep
                    inst.sync_info = si
                new.append(inst)
            insts[:] = new
    return n


def build_nc(rows=R):
    assert rows % (2 * WIN) == 0
    nwin = rows // WIN

    nc = bass.Bass()
    xha = nc.dram_tensor("xha", [nwin, K_AUG, NPAIR * P], F16, kind="ExternalInput")
    xhb = nc.dram_tensor("xhb", [nwin, K_AUG, NPAIR * P], F16, kind="ExternalInput")
    c_in = nc.dram_tensor(
        "c_in", [nwin // 2, P, 2 * CH * H_DIM], F16, kind="ExternalInput"
    )
    w = nc.dram_tensor("w", [P, 2 * G4], F16, kind="ExternalInput")
    hn_out = nc.dram_tensor(
        "hn", [nwin // 2, P, 2 * CH * H_DIM], F16, kind="ExternalOutput"
    )
    cn_out = nc.dram_tensor(
        "cn", [nwin // 2, P, 2 * CH * H_DIM], F16, kind="ExternalOutput"
    )

    with tile.TileContext(nc) as tc:
        with (
            tc.tile_pool(name="const", bufs=1) as constp,
            tc.tile_pool(name="io", bufs=3) as iop,
            tc.tile_pool(name="pair", bufs=2) as pairp,
            tc.tile_pool(name="work", bufs=3) as workp,
            tc.tile_pool(name="psum", bufs=2, space="PSUM") as psump,
        ):
            w_sb = constp.tile([P, 2 * G4], F16, tag="w")
            nc.sync.dma_start(w_sb[:], w[:])

            c_t = None
            hn_t = None
            cn_t = None
            for it in range(nwin):
                half = it % 2
                xh_sb = iop.tile([P, NPAIR, P], F16, tag="xh")
                nc.sync.dma_start(
                    xh_sb[0:K_AUG], xha[it].rearrange("k (j p) -> k j p", j=NPAIR)
                )
                nc.sync.dma_start(
                    xh_sb[K_AUG : 2 * K_AUG],
                    xhb[it].rearrange("k (j p) -> k j p", j=NPAIR),
                )
                if half == 0:
                    c_t = pairp.tile([P, 2, NPAIR, 2, H_DIM], F16, tag="c")
                    nc.sync.dma_start(
                        c_t[:].rearrange("p a j m h -> p (a j m h)"), c_in[it // 2]
                    )
                    hn_t = pairp.tile([P, 2, NPAIR, 2, H_DIM], F16, tag="hn")
                    cn_t = pairp.tile([P, 2, NPAIR, 2, H_DIM], F16, tag="cn")

                ps = psump.tile([P, NPAIR, 2 * G4], F32, tag="ps")
                for j in range(NPAIR):
                    nc.tensor.matmul(
                        ps[:, j, :],
                        xh_sb[0 : 2 * K_AUG, j, :],
                        w_sb[0 : 2 * K_AUG],
                        start=True,
                        stop=True,
                    )

                # sfo free layout per pair j: [Ai|Af|Ao|Bi|Bf|Bo] (6*32)
                sfo = workp.tile([P, NPAIR, 6 * H_DIM], F16, tag="sfo")
                nc.scalar.activation(sfo[:], ps[:, :, 0 : 6 * H_DIM], AF.Sigmoid)
                g_sb = workp.tile([P, NPAIR, 2 * H_DIM], F16, tag="g")
                nc.scalar.activation(
                    g_sb[:], ps[:, :, 6 * H_DIM : 2 * G4], AF.Tanh
                )

                sfo4 = sfo[:].rearrange("p j (m g) -> p j m g", m=2)
                g4 = g_sb[:].rearrange("p j (m h) -> p j m h", m=2)
                c4 = c_t[:, half]
                m1 = workp.tile([P, NPAIR, 2, H_DIM], F16, tag="m1")
                nc.vector.tensor_mul(m1[:], sfo4[:, :, :, 0:H_DIM], g4)
                m2 = workp.tile([P, NPAIR, 2, H_DIM], F16, tag="m2")
                nc.vector.tensor_mul(m2[:], sfo4[:, :, :, H_DIM : 2 * H_DIM], c4)
                nc.vector.tensor_add(cn_t[:, half], m1[:], m2[:])
                tc_sb = workp.tile([P, NPAIR, 2, H_DIM], F16, tag="tc")
                nc.scalar.activation(tc_sb[:], cn_t[:, half], AF.Tanh)
                nc.vector.tensor_mul(
                    hn_t[:, half], sfo4[:, :, :, 2 * H_DIM : 3 * H_DIM], tc_sb[:]
                )

                if half == 1:
                    nc.gpsimd.dma_start(
                        cn_out[it // 2], cn_t[:].rearrange("p a j m h -> p (a j m h)")
                    )
                    nc.gpsimd.dma_start(
                        hn_out[it // 2], hn_t[:].rearrange("p a j m h -> p (a j m h)")
                    )

    _split_waits(nc)
    return nc


def host_prep(x, h, c, Wx, Wh, b):
    """Layout-only host prep for the full batch. Returns fp16 arrays."""
    n = x.shape[0]
    nwin = n // WIN

    A = np.empty((n, K_AUG), dtype=np.float16)
    A[:, 0:IN_DIM] = np.asarray(x, np.float32)
    A[:, IN_DIM:XH] = np.asarray(h, np.float32)
    A[:, XH] = 1.0
    # row = w2*4096 + 32p + 16a + m,  m = 2j + half
    A6 = A.reshape(nwin // 2, P, 2, NPAIR, 2, K_AUG)  # [w2, p, a, j, half, k]
    # window w = 2*w2 + a; col index = j*128 + p
    xha = np.ascontiguousarray(
        A6[:, :, :, :, 0, :].transpose(0, 2, 4, 3, 1).reshape(nwin, K_AUG, NPAIR * P)
    )
    xhb = np.ascontiguousarray(
        A6[:, :, :, :, 1, :].transpose(0, 2, 4, 3, 1).reshape(nwin, K_AUG, NPAIR * P)
    )

    c_host = np.ascontiguousarray(
        np.asarray(c, np.float32)
        .astype(np.float16)
        .reshape(nwin // 2, P, 2 * CH * H_DIM)
    )

    W = np.concatenate(
        [np.asarray(Wx), np.asarray(Wh), np.asarray(b)[None, :]], axis=0
    ).astype(np.float32)  # [49, 128] cols [i|f|g|o]
    Wi, Wf, Wg, Wo = W[:, 0:32], W[:, 32:64], W[:, 64:96], W[:, 96:128]
    Wifo = np.concatenate([Wi, Wf, Wo], axis=1)  # [49, 96]
    w_host = np.zeros((P, 2 * G4), dtype=np.float16)
    w_host[0:K_AUG, 0:96] = Wifo  # A: i f o
    w_host[K_AUG : 2 * K_AUG, 96:192] = Wifo  # B: i f o
    w_host[0:K_AUG, 192:224] = Wg  # A: g
    w_host[K_AUG : 2 * K_AUG, 224:256] = Wg  # B: g
    return xha, xhb, c_host, w_host


_NC_CACHE = {}


def _get_nc(rows=R):
    if rows not in _NC_CACHE:
        _NC_CACHE[rows] = build_nc(rows)
    return _NC_CACHE[rows]


def run(x, h, c, Wx, Wh, b, trace=False, rows=R, n_cores=N_CORES):
    """Shard, execute on the 8 cores, gather. Returns (h_new, c_new, results)."""
    from concourse.bass_utils import run_bass_kernel_spmd

    xha, xhb, c_host, w_host = host_prep(x, h, c, Wx, Wh, b)
    nc = _get_nc(rows)
    nwin = rows // WIN
    in_maps = []
    for i in range(n_cores):
        sl = slice(i * nwin, (i + 1) * nwin)
        sl2 = slice(i * nwin // 2, (i + 1) * nwin // 2)
        in_maps.append(
            {
                "xha": xha[sl],
                "xhb": xhb[sl],
                "c_in": c_host[sl2],
                "w": w_host,
            }
        )
    res = run_bass_kernel_spmd(nc, in_maps, list(range(n_cores)), trace=trace)
    n = rows * n_cores
    h_new = np.empty((n, H_DIM), dtype=np.float32)
    c_new = np.empty((n, H_DIM), dtype=np.float32)
    for i, r in enumerate(res.results):
        sl = slice(i * rows, (i + 1) * rows)
        h_new[sl] = r["hn"].reshape(rows, H_DIM).astype(np.float32)
        c_new[sl] = r["cn"].reshape(rows, H_DIM).astype(np.float32)
    return h_new, c_new, res


def kernel(x, h, c, Wx, Wh, b):
    h_new, c_new, _ = run(x, h, c, Wx, Wh, b)
    return h_new, c_new


# revision 15
# speedup vs baseline: 1.0304x; 1.0047x over previous
"""v7: row-major LSTM cell kernel, fp16 I/O, K-stacked pair matmuls.

Perf history (HW exec, 8 cores): v3 feature-major baseline 550us ->
v4 row-major fp16 261us -> v5 pair-matmuls + DMA batching 199us.
Span is ScalarE-bound (~170us of sigmoid/tanh at 1 elem/lane/cycle --
the 5 transcendentals per output element are the hard floor; GpSimd has
no LUT and PSUM is PE-write/ACT-read only, so no further offload).
bf16 matmul inputs were tried (196us) but push rel err to 1.5e-2 vs
the 2e-2 gate; fp16 keeps it at 2e-3.

Sharding: pure data-parallel, batch split 8 ways (131072 rows/core).

Row mapping (per core): row = w2*4096 + 32p + 16a + m, where w2 = window
pair, a = window parity, p = partition, m = chunk-in-window = 2j + half
(j = matmul pair, half = A/B slot). c/hn/cn host arrays are NATURAL
reshapes [nwin/2, 128, 2*16*32] of the row-major [R, 32] arrays.

Host prep (layout only; all compute on device):
  W_pad [128,256] fp16, cols [Ai|Af|Ao|Bi|Bf|Bo|Ag|Bg] (A rows 0:49,
    B rows 64:113 of the stationary; zeros elsewhere kill junk lanes)
  xha/xhb [nwin, 49, 8*128] fp16: window w=2*w2+a, col j*128+p =
    [x|h|1](row(w2, a, p, m=2j+half)), half=0 for xha, 1 for xhb

Device, per 2048-row window:
  - xh_sb [128, 8, 128]: parts 0:49 <- xha, 64:113 <- xhb
  - 8 matmuls: lhsT = xh_sb[:, j, :] stationary, rhs = W_pad [128, 256]
    -> PSUM [128, 8, 256] f32
  - ACT: sigmoid PSUM[:, :, 0:192] -> sfo fp16; tanh [:, :, 192:256] -> g
  - DVE (fp16 SBUF 2x): m1=i*g, m2=f*c, cn=m1+m2, hn=o*tanh(cn)
  - ACT: tc=tanh(cn)
  - c in on sync; hn/cn out on gpsimd, batched per 2 windows
"""

import sys

if "/opt/trn_rl_repo" not in sys.path:
    sys.path.insert(0, "/opt/trn_rl_repo")

import ml_dtypes
import numpy as np

import bass_rust
import concourse.bass as bass
import concourse.tile as tile
from concourse import mybir

F32 = mybir.dt.float32
F16 = mybir.dt.float16
BF16 = mybir.dt.bfloat16
AF = mybir.ActivationFunctionType

B = 1048576
N_CORES = 8
R = B // N_CORES
IN_DIM, H_DIM = 16, 32
XH = IN_DIM + H_DIM
K_AUG = XH + 1  # 49
G4 = 4 * H_DIM  # 128
P = 128
NPAIR = 8  # matmul pairs per window
CH = 2 * NPAIR  # 16 chunks (of 128 rows) per window
WIN = CH * P  # 2048 rows per window
NWIN = R // WIN  # 64


def _split_waits(nc, max_waits=1):
    """Walrus codegen allows at most one semaphore wait per instruction."""
    n = 0
    for f in nc.m.functions:
        for blk in f.blocks:
            insts = blk.instructions
            new = []
            for inst in insts:
                si = inst.sync_info
                waits = list(si.on_wait) if si and si.on_wait else []
                if len(waits) > max_waits:
                    excess, keep = waits[:-max_waits], waits[-max_waits:]
                    for j in range(0, len(excess), max_waits):
                        nop = mybir.InstEventSemaphore(
                            name=f"{inst.name}-tw{j}", ins=[], outs=[]
                        )
                        nop.engine = inst.engine
                        nop.sync_info = bass_rust.SyncInfo(
                            on_wait=excess[j : j + max_waits], on_update=[]
                        )
                        new.append(nop)
                        n += 1
                    si.on_wait = keep
                    inst.sync_info = si
                new.append(inst)
            insts[:] = new
    return n


def build_nc(rows=R):
    assert rows % (2 * WIN) == 0
    nwin = rows // WIN

    nc = bass.Bass()
    xha = nc.dram_tensor("xha", [nwin, K_AUG, NPAIR * P], F16, kind="ExternalInput")
    xhb = nc.dram_tensor("xhb", [nwin, K_AUG, NPAIR * P], F16, kind="ExternalInput")
    c_in = nc.dram_tensor(
        "c_in", [nwin // 2, P, 2 * CH * H_DIM], F16, kind="ExternalInput"
    )
    w = nc.dram_tensor("w", [P, 2 * G4], F16, kind="ExternalInput")
    hn_out = nc.dram_tensor(
        "hn", [nwin // 2, P, 2 * CH * H_DIM], F16, kind="ExternalOutput"
    )
    cn_out = nc.dram_tensor(
        "cn", [nwin // 2, P, 2 * CH * H_DIM], F16, kind="ExternalOutput"
    )

    with tile.TileContext(nc) as tc:
        with (
            tc.tile_pool(name="const", bufs=1) as constp,
            tc.tile_pool(name="io", bufs=3) as iop,
            tc.tile_pool(name="pair", bufs=2) as pairp,
            tc.tile_pool(name="work", bufs=3) as workp,
            tc.tile_pool(name="psum", bufs=2, space="PSUM") as psump,
        ):
            w_sb = constp.tile([P, 2 * G4], F16, tag="w")
            nc.sync.dma_start(w_sb[:], w[:])

            c_t = None
            hn_t = None
            cn_t = None
            for it in range(nwin):
                half = it % 2
                xh_sb = iop.tile([P, NPAIR, P], F16, tag="xh")
                nc.sync.dma_start(
                    xh_sb[0:K_AUG], xha[it].rearrange("k (j p) -> k j p", j=NPAIR)
                )
                nc.sync.dma_start(
                    xh_sb[K_AUG : 2 * K_AUG],
                    xhb[it].rearrange("k (j p) -> k j p", j=NPAIR),
                )
                if half == 0:
                    c_t = pairp.tile([P, 2, NPAIR, 2, H_DIM], F16, tag="c")
                    nc.sync.dma_start(
                        c_t[:].rearrange("p a j m h -> p (a j m h)"), c_in[it // 2]
                    )
                    hn_t = pairp.tile([P, 2, NPAIR, 2, H_DIM], F16, tag="hn")
                    cn_t = pairp.tile([P, 2, NPAIR, 2, H_DIM], F16, tag="cn")

                ps = psump.tile([P, NPAIR, 2 * G4], F32, tag="ps")
                for j in range(NPAIR):
                    nc.tensor.matmul(
                        ps[:, j, :],
                        xh_sb[0 : 2 * K_AUG, j, :],
                        w_sb[0 : 2 * K_AUG],
                        start=True,
                        stop=True,
                    )

                # sfo free layout per pair j: [Ai|Af|Ao|Bi|Bf|Bo] (6*32)
                sfo = workp.tile([P, NPAIR, 6 * H_DIM], F16, tag="sfo")
                nc.scalar.activation(sfo[:], ps[:, :, 0 : 6 * H_DIM], AF.Sigmoid)
                g_sb = workp.tile([P, NPAIR, 2 * H_DIM], F16, tag="g")
                nc.scalar.activation(g_sb[:], ps[:, :, 6 * H_DIM : 2 * G4], AF.Tanh)

                sfo4 = sfo[:].rearrange("p j (m g) -> p j m g", m=2)
                g4 = g_sb[:].rearrange("p j (m h) -> p j m h", m=2)
                c4 = c_t[:, half]
                m1 = workp.tile([P, NPAIR, 2, H_DIM], F16, tag="m1")
                nc.vector.tensor_mul(m1[:], sfo4[:, :, :, 0:H_DIM], g4)
                m2 = workp.tile([P, NPAIR, 2, H_DIM], F16, tag="m2")
                nc.vector.tensor_mul(m2[:], sfo4[:, :, :, H_DIM : 2 * H_DIM], c4)
                nc.vector.tensor_add(cn_t[:, half], m1[:], m2[:])
                tc_sb = workp.tile([P, NPAIR, 2, H_DIM], F16, tag="tc")
                nc.scalar.activation(tc_sb[:], cn_t[:, half], AF.Tanh)
                nc.vector.tensor_mul(
                    hn_t[:, half], sfo4[:, :, :, 2 * H_DIM : 3 * H_DIM], tc_sb[:]
                )

                if half == 1:
                    nc.gpsimd.dma_start(
                        cn_out[it // 2], cn_t[:].rearrange("p a j m h -> p (a j m h)")
                    )
                    nc.gpsimd.dma_start(
                        hn_out[it // 2], hn_t[:].rearrange("p a j m h -> p (a j m h)")
                    )

    _split_waits(nc)
    return nc


def host_prep(x, h, c, Wx, Wh, b):
    """Layout-only host prep for the full batch. Returns fp16 arrays."""
    n = x.shape[0]
    nwin = n // WIN

    A = np.empty((n, K_AUG), dtype=np.float16)
    A[:, 0:IN_DIM] = np.asarray(x, np.float32)
    A[:, IN_DIM:XH] = np.asarray(h, np.float32)
    A[:, XH] = 1.0
    # row = w2*4096 + 32p + 16a + m,  m = 2j + half
    A6 = A.reshape(nwin // 2, P, 2, NPAIR, 2, K_AUG)  # [w2, p, a, j, half, k]
    # window w = 2*w2 + a; col index = j*128 + p
    xha = np.ascontiguousarray(
        A6[:, :, :, :, 0, :].transpose(0, 2, 4, 3, 1).reshape(nwin, K_AUG, NPAIR * P)
    )
    xhb = np.ascontiguousarray(
        A6[:, :, :, :, 1, :].transpose(0, 2, 4, 3, 1).reshape(nwin, K_AUG, NPAIR * P)
    )

    c_host = np.ascontiguousarray(
        np.asarray(c, np.float32)
        .astype(np.float16)
        .reshape(nwin // 2, P, 2 * CH * H_DIM)
    )

    W = np.concatenate(
        [np.asarray(Wx), np.asarray(Wh), np.asarray(b)[None, :]], axis=0
    ).astype(np.float32)  # [49, 128] cols [i|f|g|o]
    Wi, Wf, Wg, Wo = W[:, 0:32], W[:, 32:64], W[:, 64:96], W[:, 96:128]
    Wifo = np.concatenate([Wi, Wf, Wo], axis=1)  # [49, 96]
    w_host = np.zeros((P, 2 * G4), dtype=np.float16)
    w_host[0:K_AUG, 0:96] = Wifo  # A: i f o
    w_host[K_AUG : 2 * K_AUG, 96:192] = Wifo  # B: i f o
    w_host[0:K_AUG, 192:224] = Wg  # A: g
    w_host[K_AUG : 2 * K_AUG, 224:256] = Wg  # B: g
    return xha, xhb, c_host, w_host


_NC_CACHE = {}


def _get_nc(rows=R):
    if rows not in _NC_CACHE:
        _NC_CACHE[rows] = build_nc(rows)
    return _NC_CACHE[rows]


def run(x, h, c, Wx, Wh, b, trace=False, rows=R, n_cores=N_CORES):
    """Shard, execute on the 8 cores, gather. Returns (h_new, c_new, results)."""
    from concourse.bass_utils import run_bass_kernel_spmd

    xha, xhb, c_host, w_host = host_prep(x, h, c, Wx, Wh, b)
    nc = _get_nc(rows)
    nwin = rows // WIN
    in_maps = []
    for i in range(n_cores):
        sl = slice(i * nwin, (i + 1) * nwin)
        sl2 = slice(i * nwin // 2, (i + 1) * nwin // 2)
        in_maps.append(
            {
                "xha": xha[sl],
                "xhb": xhb[sl],
                "c_in": c_host[sl2],
                "w": w_host,
            }
        )
    res = run_bass_kernel_spmd(nc, in_maps, list(range(n_cores)), trace=trace)
    n = rows * n_cores
    h_new = np.empty((n, H_DIM), dtype=np.float32)
    c_new = np.empty((n, H_DIM), dtype=np.float32)
    for i, r in enumerate(res.results):
        sl = slice(i * rows, (i + 1) * rows)
        h_new[sl] = r["hn"].reshape(rows, H_DIM).astype(np.float32)
        c_new[sl] = r["cn"].reshape(rows, H_DIM).astype(np.float32)
    return h_new, c_new, res


def kernel(x, h, c, Wx, Wh, b):
    h_new, c_new, _ = run(x, h, c, Wx, Wh, b)
    return h_new, c_new
